# revision 13
# baseline (speedup 1.0000x reference)
"""Trainium2 Bass kernel for a bilinear cross-attention dual-stream block.

Reference computation (B=2, L=2048, D=1024, H=16 heads, HD=64, R=16):
    h_seq    = BilinearXAttn(LN(x_seq; g_s, b_s),  x_struct, seq_*)
    x_seq    = x_seq + h_seq
    h_struct = BilinearXAttn(LN(x_struct; g_t, b_t), x_seq,  st_*)
    x_struct = x_struct + h_struct
    return (x_seq, x_struct)

where BilinearXAttn(q_in, kv_in):
    scores[b,h,q,k] = (q_in @ Wq + bq)U_h . (kv_in @ Wk + bk)V_h / sqrt(R)
    out = softmax(scores) @ (kv_in @ Wv + bv) ; out @ Wo + bo

Key algebraic folds done on the host (pure weight reparameterization; all
activation-dependent work runs on device):
  * q/k are never materialized: ql = LN(x) @ A + a with A = diag(g)(Wq U)/sqrt(R),
    a = (b_ln (Wq U) + bq U)/sqrt(R); kl = kv @ Bm + bm with Bm = Wk V, bm = bk V.
  * bv folds into bo (softmax rows sum to 1): bo_eff = bo + bv @ Wo.

Sharding (8 cores): DP-2 over batch x sequence-parallel-4 over query rows.
Cores 4b..4b+3 handle batch b; core owns LQ=512 query rows. KV-side tensors
(kl, v) are computed redundantly per core from locally available full inputs,
which makes block 1 collective-free. The single collective is an AllGather of
the updated x_seq (block 1 output) within each batch group, which block 2
consumes as its KV stream.

Device attention layout: scores are built transposed, S^T[k, q] (k on
partitions), so the PV matmul needs no transposition of the probability
matrix. Softmax runs without max-subtraction (scores here are ~N(0, 0.05^2);
exp is safe in fp32). The softmax denominator is obtained for free as an
extra output row of the PV matmul by appending a ones-column to V.
"""

import os
import sys

sys.path.insert(0, "/opt/trn_rl_repo")

import numpy as np
from contextlib import ExitStack

import concourse.bass as bass
import concourse.tile as tile
from concourse import bacc, mybir
from concourse.bass_utils import run_bass_kernel_spmd
from concourse.masks import make_identity

F32 = mybir.dt.float32
BF16 = mybir.dt.bfloat16
AF = mybir.ActivationFunctionType
ALU = mybir.AluOpType

B, L, D, H, R, HD = 2, 2048, 1024, 16, 16, 64
RP = 32             # rank rows per head, zero-padded 16->32 (PE row groups
                    # are 32-aligned; matmul base partitions must be 0/32/64)
GH = 3              # heads per 128-partition group (bases 0/32/64 only)
NG = 6              # ceil(H/GH) partition groups
HR = NG * 128       # 768 packed (padded) rank rows
MH = NG
HDA = HD + 1        # v columns per head + ones column (denominator row)
EPS = 1e-5
NCORES = 8
GP = 4              # cores per batch group
LQ = L // GP        # query rows owned per core = 512
KD = D // 128       # 8 contraction tiles over D
KT = L // 128       # 16 contraction tiles over L (keys)
QT = LQ // 128      # 4 query subtiles
REPLICA_GROUPS = [[0, 1, 2, 3], [4, 5, 6, 7]]

_CACHE = {}
LAST_RESULTS = None  # BassKernelResults of the most recent run (for test.py)


# --------------------------------------------------------------------------
# device kernel
# --------------------------------------------------------------------------

def _block(tc, cst, xq, xkv, W, out_dram, cc_in, tag):
    """One bilinear cross-attention block for the owned query rows.

    Single pool scope: projections, attention, and out-projection share
    the scheduler window so PE work (v projection, PV) can fill the gaps
    left by the ACT-bound exp stream. PSUM budget (8 banks): mix(tr+pv
    1 bank x2) + pj(1 bank x2) + sp(4 banks x1).
    """
    nc = tc.nc
    ident = cst["id32"] if xkv.dtype == F32 else cst["id16"]
    with ExitStack() as blk:
        # ---- pools (stack order matters: long-lived first) ----
        sb = blk.enter_context(tc.tile_pool(name=f"sb{tag}", bufs=1))
        ep = blk.enter_context(tc.tile_pool(name=f"ep{tag}", bufs=2))
        lw = blk.enter_context(tc.tile_pool(name=f"lw{tag}", bufs=1))
        work = blk.enter_context(tc.tile_pool(name=f"wk{tag}", bufs=2))
        stp = blk.enter_context(tc.tile_pool(name=f"st{tag}", bufs=3))
        rp = blk.enter_context(tc.tile_pool(name=f"rp{tag}", bufs=2))
        wpe = blk.enter_context(tc.tile_pool(name=f"we{tag}", bufs=1))
        mix_ps = blk.enter_context(tc.tile_pool(name=f"mx{tag}", bufs=2,
                                                space="PSUM"))
        pj_ps = blk.enter_context(tc.tile_pool(name=f"pj{tag}", bufs=2,
                                               space="PSUM"))
        s_ps = blk.enter_context(tc.tile_pool(name=f"sp{tag}", bufs=1,
                                              space="PSUM"))

        # ---- persistent tiles ----
        qlT = sb.tile([128, MH, LQ], BF16, name=f"qlT{tag}")
        klT = sb.tile([128, MH, L], BF16, name=f"klT{tag}")
        v_aug = sb.tile([128, KT, H, HDA], BF16, name=f"vaug{tag}")
        attn_outT = sb.tile([128, KD, LQ], BF16, name=f"aoT{tag}")
        a_sb = sb.tile([128, MH], F32, name=f"a{tag}")
        nc.sync.dma_start(a_sb[:], W["a"][:])
        b_sb = sb.tile([128, MH], F32, name=f"b{tag}")
        nc.sync.dma_start(b_sb[:], W["b"][:])
        use_bo = W["bo"] is not None
        bo_sb = None
        if use_bo:
            bo_sb = sb.tile([128, D], F32, name=f"bo{tag}")
            bo_b = W["bo"]
            nc.sync.dma_start(
                bo_sb[:],
                bass.AP(tensor=bo_b.tensor, offset=bo_b.offset,
                        ap=[[0, 128]] + list(bo_b.ap[1:])))
        # ones column of v_aug feeds the denominator row of the PV matmul
        nc.vector.memset(v_aug[:, :, :, HD:HDA], 1.0)

        Wo_sb = lw.tile([128, KD, D], BF16, name=f"Wo{tag}")
        nc.sync.dma_start(Wo_sb[:], W["Wo"].rearrange("(k p) m -> p k m",
                                                      p=128))
        A_sb = wpe.tile([128, KD, HR], BF16, name=f"A{tag}")
        nc.sync.dma_start(A_sb[:], W["A"].rearrange("(k p) m -> p k m", p=128))
        B_sb = wpe.tile([128, KD, HR], BF16, name=f"B{tag}")
        nc.sync.dma_start(B_sb[:], W["B"].rearrange("(k p) m -> p k m", p=128))
        Wv_sb = wpe.tile([128, KD, D], BF16, name=f"Wv{tag}")
        nc.sync.dma_start(Wv_sb[:], W["Wv"].rearrange("(k p) m -> p k m",
                                                      p=128))
        lnqT = wpe.tile([128, KD, LQ], BF16, name=f"lnqT{tag}")

        # ---- q side: LN -> transpose -> rank projection ----
        for t in range(QT):
            xt = work.tile([128, D], F32, tag="xt")
            nc.sync.dma_start(xt[:], xq[t * 128:(t + 1) * 128, :])
            xv = xt.rearrange("p (s f) -> p s f", f=512)
            stats = stp.tile([128, 2, 6], F32, tag="stats")
            for s in range(2):
                nc.vector.bn_stats(out=stats[:, s, :], in_=xv[:, s, :])
            mv = stp.tile([128, 2], F32, tag="mv")
            nc.vector.bn_aggr(out=mv[:], in_=stats[:])
            rstd = stp.tile([128, 1], F32, tag="rstd")
            nc.scalar.activation(out=rstd[:], in_=mv[:, 1:2], func=AF.Sqrt,
                                 bias=cst["eps"][:], scale=1.0)
            nc.vector.reciprocal(out=rstd[:], in_=rstd[:])
            z = work.tile([128, D], BF16, tag="z")
            nc.vector.tensor_scalar(out=z[:], in0=xt[:], scalar1=mv[:, 0:1],
                                    scalar2=rstd[:], op0=ALU.subtract,
                                    op1=ALU.mult)
            for d in range(KD):
                pt = mix_ps.tile([128, 128], BF16, tag="mix")
                nc.tensor.transpose(pt[:], z[:, d * 128:(d + 1) * 128],
                                    cst["id16"][:])
                nc.scalar.copy(out=lnqT[:, d, t * 128:(t + 1) * 128],
                               in_=pt[:])
        for mh in range(MH):
            ps = pj_ps.tile([128, LQ], F32, tag="pj")
            for k in range(KD):
                nc.tensor.matmul(ps[:], A_sb[:, k, mh * 128:(mh + 1) * 128],
                                 lnqT[:, k, :], start=(k == 0),
                                 stop=(k == KD - 1))
            nc.vector.tensor_scalar(out=qlT[:, mh, :], in0=ps[:],
                                    scalar1=a_sb[:, mh:mh + 1], scalar2=None,
                                    op0=ALU.add)

        # ---- kv side: transpose chunks -> kl projection -> v projection ----
        for c in range(KT // 4):          # 4 chunks of 512 key rows
            xkvT = work.tile([128, KD, 512], BF16, tag="xkvT", bufs=1)
            for m in range(4):
                kt = c * 4 + m
                xt = work.tile([128, D], xkv.dtype, tag="xt")
                nc.sync.dma_start(xt[:], xkv[kt * 128:(kt + 1) * 128, :])
                for d in range(KD):
                    pt = mix_ps.tile([128, 128], xkv.dtype, tag="mix")
                    nc.tensor.transpose(pt[:], xt[:, d * 128:(d + 1) * 128],
                                        ident[:])
                    nc.scalar.copy(out=xkvT[:, d, m * 128:(m + 1) * 128],
                                   in_=pt[:])
            for mh in range(MH):
                ps = pj_ps.tile([128, 512], F32, tag="pj")
                for k in range(KD):
                    nc.tensor.matmul(ps[:], B_sb[:, k, mh * 128:(mh + 1) * 128],
                                     xkvT[:, k, :], start=(k == 0),
                                     stop=(k == KD - 1))
                nc.vector.tensor_scalar(out=klT[:, mh, c * 512:(c + 1) * 512],
                                        in0=ps[:], scalar1=b_sb[:, mh:mh + 1],
                                        scalar2=None, op0=ALU.add)
            for m in range(4):
                kt = c * 4 + m
                for nh in range(2):
                    pv = pj_ps.tile([128, 512], F32, tag="pj")
                    for k in range(KD):
                        nc.tensor.matmul(
                            pv[:], xkvT[:, k, m * 128:(m + 1) * 128],
                            Wv_sb[:, k, nh * 512:(nh + 1) * 512],
                            start=(k == 0), stop=(k == KD - 1))
                    nc.vector.tensor_copy(
                        out=v_aug[:, kt, nh * 8:(nh + 1) * 8, 0:HD],
                        in_=pv.rearrange("p (h d) -> p h d", d=HD))

        # ---- attention: S^T -> exp -> PV (+denominator row) -> normalize --
        for h in range(H):
            mh, poff = h // GH, (h % GH) * RP
            expS = ep.tile([128, KT, LQ], BF16, tag="expS")
            for g in range(KT // 4):
                ps = s_ps.tile([128, 4, LQ], F32, tag="sp")
                for kk in range(4):
                    kt = g * 4 + kk
                    nc.tensor.matmul(ps[:, kk, :],
                                     klT[poff:poff + RP, mh,
                                         kt * 128:(kt + 1) * 128],
                                     qlT[poff:poff + RP, mh, :],
                                     start=True, stop=True)
                nc.scalar.activation(out=expS[:, 4 * g:4 * g + 4, :],
                                     in_=ps[:], func=AF.Exp)
            po = mix_ps.tile([HDA, LQ], F32, tag="mix")
            for kt in range(KT):
                nc.tensor.matmul(po[:], v_aug[:, kt, h, :], expS[:, kt, :],
                                 start=(kt == 0), stop=(kt == KT - 1))
            recip = rp.tile([1, LQ], F32, tag="recip")
            nc.vector.reciprocal(out=recip[:], in_=po[HD:HDA, :])
            bcast = rp.tile([HD, LQ], F32, tag="bcast")
            nc.gpsimd.partition_broadcast(out_ap=bcast[:], in_ap=recip[:])
            nc.vector.tensor_mul(
                out=attn_outT[(h % 2) * HD:(h % 2 + 1) * HD, h // 2, :],
                in0=po[0:HD, :], in1=bcast[:])

        # ---- out-projection + residual ----
        for mt in range(QT):
            o = work.tile([128, D], F32, tag="o")
            nc.sync.dma_start(o[:], xq[mt * 128:(mt + 1) * 128, :])
            if use_bo:
                nc.vector.tensor_add(out=o[:], in0=o[:], in1=bo_sb[:])
            for nh in range(2):
                phm = pj_ps.tile([128, 512], F32, tag="pj")
                for k in range(KD):
                    nc.tensor.matmul(phm[:],
                                     attn_outT[:, k, mt * 128:(mt + 1) * 128],
                                     Wo_sb[:, k, nh * 512:(nh + 1) * 512],
                                     start=(k == 0), stop=(k == KD - 1))
                nc.vector.tensor_add(out=o[:, nh * 512:(nh + 1) * 512],
                                     in0=phm[:],
                                     in1=o[:, nh * 512:(nh + 1) * 512])
            nc.sync.dma_start(out_dram[mt * 128:(mt + 1) * 128, :], o[:])
            if cc_in is not None:
                ob = work.tile([128, D], BF16, tag="z")
                nc.vector.tensor_copy(out=ob[:], in_=o[:])
                nc.sync.dma_start(cc_in[mt * 128:(mt + 1) * 128, :], ob[:])


def _build(use_bo1, use_bo2):
    nc = bacc.Bacc("TRN2", target_bir_lowering=False, debug=False,
                   num_devices=NCORES)

    def din(name, shape, dt=F32):
        return nc.dram_tensor(name, shape, dt, kind="ExternalInput")[:]

    xq1 = din("xq1", [LQ, D])
    xkv1 = din("xkv1", [L, D])
    xq2 = din("xq2", [LQ, D])
    W1 = {"A": din("A1", [D, HR], BF16), "a": din("a1", [128, MH]),
          "B": din("B1", [D, HR], BF16), "b": din("b1", [128, MH]),
          "Wv": din("Wv1", [D, D], BF16), "Wo": din("Wo1", [D, D], BF16),
          "bo": din("bo1", [1, D]) if use_bo1 else None}
    W2 = {"A": din("A2", [D, HR], BF16), "a": din("a2", [128, MH]),
          "B": din("B2", [D, HR], BF16), "b": din("b2", [128, MH]),
          "Wv": din("Wv2", [D, D], BF16), "Wo": din("Wo2", [D, D], BF16),
          "bo": din("bo2", [1, D]) if use_bo2 else None}
    out1 = nc.dram_tensor("out1", [LQ, D], F32, kind="ExternalOutput")[:]
    out2 = nc.dram_tensor("out2", [LQ, D], F32, kind="ExternalOutput")[:]

    with tile.TileContext(nc) as tc:
        with ExitStack() as top:
            dram = top.enter_context(tc.tile_pool(name="dram", bufs=1,
                                                  space="DRAM"))
            cc_in = dram.tile([LQ, D], BF16)
            cc_out = dram.tile([L, D], BF16)
            csts = top.enter_context(tc.tile_pool(name="csts", bufs=1))
            id32 = csts.tile([128, 128], F32)
            make_identity(nc, id32)
            id16 = csts.tile([128, 128], BF16)
            nc.vector.tensor_copy(out=id16[:], in_=id32[:])
            eps = csts.tile([128, 1], F32)
            nc.vector.memset(eps[:], EPS)
            cst = {"id32": id32, "id16": id16, "eps": eps}

            _block(tc, cst, xq1, xkv1, W1, out1, cc_in, "1")
            nc.gpsimd.collective_compute(
                "AllGather", ALU.bypass, replica_groups=REPLICA_GROUPS,
                ins=[cc_in.opt()], outs=[cc_out.opt()])
            _block(tc, cst, xq2, cc_out, W2, out2, None, "2")

    nc.compile()
    return nc


# --------------------------------------------------------------------------
# host wrapper
# --------------------------------------------------------------------------

def _fold(Wq, bq, U, Wk, bk, V, Wv, bv, Wo, bo, g, b_ln):
    """Fold projections into rank-space matrices (see module docstring)."""
    f64 = np.float64
    Wq, bq, U = Wq.astype(f64), bq.astype(f64), U.astype(f64)
    Wk, bk, V = Wk.astype(f64), bk.astype(f64), V.astype(f64)
    Wv, bv = Wv.astype(f64), bv.astype(f64)
    Wo, bo = Wo.astype(f64), bo.astype(f64)
    g, b_ln = g.astype(f64), b_ln.astype(f64)
    s = 1.0 / np.sqrt(R)
    A = np.zeros((D, HR), f64)
    a = np.zeros(HR, f64)
    Bm = np.zeros((D, HR), f64)
    bm = np.zeros(HR, f64)
    for h in range(H):
        col = (h // GH) * 128 + (h % GH) * RP
        WqU_h = Wq[:, h * HD:(h + 1) * HD] @ U[h]     # [D, R]
        A[:, col:col + R] = (g[:, None] * WqU_h) * s
        a[col:col + R] = (b_ln @ WqU_h + bq[h * HD:(h + 1) * HD] @ U[h]) * s
        WkV_h = Wk[:, h * HD:(h + 1) * HD] @ V[h]
        Bm[:, col:col + R] = WkV_h
        bm[col:col + R] = bk[h * HD:(h + 1) * HD] @ V[h]
    bo_eff = bo + bv @ Wo
    f32 = np.float32
    import ml_dtypes
    bf16 = ml_dtypes.bfloat16
    return {"A": np.ascontiguousarray(A.astype(f32), bf16),
            "a": np.ascontiguousarray(a.reshape(MH, 128).T, f32),
            "B": np.ascontiguousarray(Bm.astype(f32), bf16),
            "b": np.ascontiguousarray(bm.reshape(MH, 128).T, f32),
            "Wv": np.ascontiguousarray(Wv.astype(f32), bf16),
            "Wo": np.ascontiguousarray(Wo.astype(f32), bf16),
            "bo": np.ascontiguousarray(bo_eff.reshape(1, D), f32)}


def _host_reference(x_seq, x_struct, padding_mask, ln_seq_g, ln_seq_b,
                    ln_st_g, ln_st_b, **w):
    """Exact numpy fallback (only used if padding_mask has any True)."""
    def ln(x, g, b):
        m = x.mean(-1, keepdims=True)
        v = x.var(-1, keepdims=True)
        return (x - m) / np.sqrt(v + EPS) * g + b

    def attn(q_in, kv_in, p):
        q = (q_in @ w[p + "_Wq"] + w[p + "_bq"]).reshape(B, L, H, HD)
        k = (kv_in @ w[p + "_Wk"] + w[p + "_bk"]).reshape(B, L, H, HD)
        v = (kv_in @ w[p + "_Wv"] + w[p + "_bv"]).reshape(B, L, H, HD)
        ql = np.einsum("blhd,hdr->bhlr", q, w[p + "_U"])
        kl = np.einsum("blhd,hdr->bhlr", k, w[p + "_V"])
        s = np.einsum("bhqr,bhkr->bhqk", ql, kl) / np.sqrt(np.float32(R))
        s = np.where(padding_mask[:, None, None, :], np.float32(-1e9), s)
        s = s - s.max(-1, keepdims=True)
        e = np.exp(s)
        a = e / e.sum(-1, keepdims=True)
        o = np.einsum("bhqk,bkhd->bqhd", a, v).reshape(B, L, D)
        return o @ w[p + "_Wo"] + w[p + "_bo"]

    x_seq = x_seq + attn(ln(x_seq, ln_seq_g, ln_seq_b), x_struct, "seq")
    x_struct = x_struct + attn(ln(x_struct, ln_st_g, ln_st_b), x_seq, "st")
    return (x_seq.astype(np.float32), x_struct.astype(np.float32))


def _ensure_ntff_hook():
    """This image's antenv lacks axon_hooks; synthesize it so trace=True
    can capture NTFF profiles through libaxon_pjrt (same as trn_boot)."""
    import types
    try:
        from antenv.axon_hooks import get_axon_ntff_profile_hook  # noqa: F401
        return
    except ImportError:
        pass
    try:
        if "/root/.axon_site" not in sys.path:
            sys.path.insert(0, "/root/.axon_site")
        from trn_agent_boot.trn_boot import _ntff_profile_via_ctypes
        hook = _ntff_profile_via_ctypes("/opt/axon/libaxon_pjrt.so")
    except Exception:
        hook = None
    mod = types.ModuleType("antenv.axon_hooks")
    mod._hook = hook

    def set_axon_ntff_profile_hook(h):
        mod._hook = h

    def get_axon_ntff_profile_hook():
        return mod._hook

    mod.set_axon_ntff_profile_hook = set_axon_ntff_profile_hook
    mod.get_axon_ntff_profile_hook = get_axon_ntff_profile_hook
    import antenv
    antenv.axon_hooks = mod
    sys.modules["antenv.axon_hooks"] = mod


def kernel(**inputs):
    global LAST_RESULTS
    inp = {k: np.asarray(v) for k, v in inputs.items()}
    if inp["padding_mask"].any():
        # Spec fills the mask with zeros; exact fallback for completeness.
        return _host_reference(**inp)

    w1 = _fold(inp["seq_Wq"], inp["seq_bq"], inp["seq_U"], inp["seq_Wk"],
               inp["seq_bk"], inp["seq_V"], inp["seq_Wv"], inp["seq_bv"],
               inp["seq_Wo"], inp["seq_bo"], inp["ln_seq_g"], inp["ln_seq_b"])
    w2 = _fold(inp["st_Wq"], inp["st_bq"], inp["st_U"], inp["st_Wk"],
               inp["st_bk"], inp["st_V"], inp["st_Wv"], inp["st_bv"],
               inp["st_Wo"], inp["st_bo"], inp["ln_st_g"], inp["ln_st_b"])
    use_bo1 = bool(np.any(w1["bo"]))
    use_bo2 = bool(np.any(w2["bo"]))

    key = (use_bo1, use_bo2)
    if key not in _CACHE:
        _CACHE[key] = _build(use_bo1, use_bo2)
    nc = _CACHE[key]

    x_seq = np.ascontiguousarray(inp["x_seq"], np.float32)
    x_struct = np.ascontiguousarray(inp["x_struct"], np.float32)

    in_maps = []
    for c in range(NCORES):
        b, qi = c // GP, c % GP
        m = {"xq1": x_seq[b, qi * LQ:(qi + 1) * LQ],
             "xkv1": x_struct[b],
             "xq2": x_struct[b, qi * LQ:(qi + 1) * LQ]}
        for tag, w in (("1", w1), ("2", w2)):
            m["A" + tag] = w["A"]
            m["a" + tag] = w["a"]
            m["B" + tag] = w["B"]
            m["b" + tag] = w["b"]
            m["Wv" + tag] = w["Wv"]
            m["Wo" + tag] = w["Wo"]
            if (use_bo1 if tag == "1" else use_bo2):
                m["bo" + tag] = w["bo"]
        in_maps.append(m)

    trace = bool(int(os.environ.get("KERNEL_TRACE", "0")))
    if trace:
        _ensure_ntff_hook()
    LAST_RESULTS = run_bass_kernel_spmd(nc, in_maps, list(range(NCORES)),
                                        trace=trace)
    res = LAST_RESULTS.results

    x_seq_out = np.empty((B, L, D), np.float32)
    x_struct_out = np.empty((B, L, D), np.float32)
    for c in range(NCORES):
        b, qi = c // GP, c % GP
        x_seq_out[b, qi * LQ:(qi + 1) * LQ] = res[c]["out1"]
        x_struct_out[b, qi * LQ:(qi + 1) * LQ] = res[c]["out2"]
    return (x_seq_out, x_struct_out)
